# revision 47
# baseline (speedup 1.0000x reference)
"""Trainium2 Bass kernel for nn_MF2Net (two tiny MLPs + Choquet integral + softmax).

Strategy: pure data parallel over the batch dim (8 NeuronCores x 32768 rows).
Host pre-packs x as fp8-e4m3 in feature-major (transposed) 4-tile-group
layout (the two MLP input matmuls dominate and tolerate fp8: measured
3.0e-3 output rel err vs the 2e-2 gate), so the device streams contiguous
1 MB groups on a single HWDGE queue and feeds the PE with zero transposes
and zero PSUM->SBUF copies:
  - DMA xt [128 feat_p, 4 tiles, 4 kc, 512 rows] fp8 per group
  - mm1: pm1[128 hid, 512 r] = sum_kc w13(bf16)[:,kc,:]^T @ xt[:,u,kc,:]
  - ACT: H = relu(pm1 + b13) -> bf16
  - mm2: per 128-row group g: pm2q[:, (u%4)*32+g*8 ...] = H-chunk^T @ wcat
  - per 4-tile quad: DVE +b24, ACT sigmoid into E
  - per 16-tile batch: Choquet combine + softmax-as-sigmoid (DVE chain),
    output written as [128, 128] f32, host un-permutes.
Software-pipelined with stage lags (mm1/relu lag 1, mm2 lag 3, quad lag 8,
epilogue lags 10/12) so the PE stays busy and the sync queue (DMA issues +
semaphore broadcasts) never head-of-line blocks; E/pr/ob live in one merged
per-batch tile to cut semaphore count. Steady state ~0.96 us per 512-row
tile, bound by the tensor+scalar chain, DMA well below its ~400 GB/s/core
ceiling.
"""
import numpy as np
import ml_dtypes
from contextlib import ExitStack

import concourse.bass as bass
import concourse.bacc as bacc
import concourse.tile as tile
import concourse.mybir as mybir
from concourse import bass_utils

N_CORES = 8
B = 262144
D = 512
R = B // N_CORES            # rows per core = 32768
TILE_ROWS = 512
N_TILES = R // TILE_ROWS    # 64
TILES_PER_BATCH = 16
BATCH_ROWS = TILE_ROWS * TILES_PER_BATCH   # 8192
N_BATCH = N_TILES // TILES_PER_BATCH       # 4
TG = TILES_PER_BATCH * 4                   # 64 (tile16, group) pairs per batch

_CACHE = {}


def _build():
    f32 = mybir.dt.float32
    bf16 = mybir.dt.bfloat16
    AF = mybir.ActivationFunctionType
    OP = mybir.AluOpType

    nc = bacc.Bacc("TRN2", target_bir_lowering=False, debug=False,
                   enable_asserts=False, num_devices=N_CORES)
    fp8 = mybir.dt.float8e4
    # x transposed+tiled on host, 4-tile chunks: [chunk, feat_part, tile, kc*rows]
    xT_d = nc.dram_tensor("xT", [N_TILES // 4, 128, 4, 4 * TILE_ROWS], fp8,
                          kind="ExternalInput").ap()
    # probs packed on host: [batch, row_part, tile16*group*4] f32
    probs_d = nc.dram_tensor("probs", [N_BATCH, 128, TG * 4], f32,
                             kind="ExternalInput").ap()
    w13_d = nc.dram_tensor("w13", [D, 128], bf16, kind="ExternalInput").ap()
    wcat_d = nc.dram_tensor("wcat", [128, 8], bf16, kind="ExternalInput").ap()
    b13_d = nc.dram_tensor("b13", [128, 1], f32, kind="ExternalInput").ap()
    b24_d = nc.dram_tensor("b24", [128, 128], f32, kind="ExternalInput").ap()
    # out packed: [batch, row_part, tile16*group*2] f32 (host un-permutes)
    out_d = nc.dram_tensor("out", [N_BATCH, 128, TG * 2], f32,
                           kind="ExternalOutput").ap()

    with tile.TileContext(nc) as tc, ExitStack() as ctx:
        wpool = ctx.enter_context(tc.tile_pool(name="w", bufs=1))
        xtp = ctx.enter_context(tc.tile_pool(name="xt", bufs=6))
        hp = ctx.enter_context(tc.tile_pool(name="h", bufs=4))
        epool = ctx.enter_context(tc.tile_pool(name="e", bufs=2))
        tpool = ctx.enter_context(tc.tile_pool(name="t", bufs=2))
        pm1p = ctx.enter_context(tc.tile_pool(name="pm1", bufs=4, space="PSUM"))
        pm2p = ctx.enter_context(tc.tile_pool(name="pm2", bufs=3, space="PSUM"))

        W = {}

        def load_weights():
            W["w13"] = wpool.tile([128, 4, 128], bf16, name="w13sb")
            nc.sync.dma_start(W["w13"][:], w13_d.rearrange("(c p) h -> p c h", p=128))
            W["wcat"] = wpool.tile([128, 8], bf16, name="wcatsb")
            nc.sync.dma_start(W["wcat"][:], wcat_d)
            W["b13"] = wpool.tile([128, 1], f32, name="b13sb")
            nc.sync.dma_start(W["b13"][:], b13_d)
            W["b24"] = wpool.tile([128, 128], f32, name="b24sb")
            nc.sync.dma_start(W["b24"][:], b24_d)

        def epi_chain(eng, Eb, prb, sc, kc, use_pred):
            E4 = Eb.rearrange("q t (g c) -> q (t g) c", c=8)
            P4 = prb.rearrange("q (tg i) -> q tg i", i=4)
            mu1 = E4[:, :, 0 + kc]
            mu2 = E4[:, :, 2 + kc]
            inc = E4[:, :, 4 + kc]
            p0 = P4[:, :, 0 + kc]
            p1 = P4[:, :, 2 + kc]
            o = kc * (5 * TG + 16)
            mx = sc[:, o:o + TG]
            dm = sc[:, o + TG:o + 2 * TG]
            r1 = sc[:, o + 2 * TG:o + 3 * TG]
            r2 = sc[:, o + 3 * TG:o + 4 * TG]
            rs = sc[:, o + 4 * TG:o + 5 * TG]
            msk = sc[:, o + 5 * TG:o + 5 * TG + 16].bitcast(mybir.dt.uint8)
            eng.tensor_tensor(mx, mu1, mu2, OP.max)
            eng.tensor_tensor(mx, mx, inc, OP.add)
            eng.tensor_scalar_min(mx, mx, 1.0)
            eng.tensor_tensor(dm, p1, p0, OP.subtract)
            eng.tensor_tensor(dm, dm, mx, OP.mult)
            eng.tensor_tensor(r1, p0, mu1, OP.mult)
            eng.tensor_tensor(r1, r1, dm, OP.add)
            eng.tensor_tensor(r2, p1, mu2, OP.mult)
            eng.tensor_tensor(r2, r2, dm, OP.subtract)
            eng.tensor_tensor(msk, p0, p1, OP.is_le)
            eng.tensor_copy(rs, r2)
            eng.copy_predicated(rs, msk, r1)
            return rs

        # Software-pipelined stages (lags keep the PE continuously busy so it
        # ramps to its max p-state): dma(t) | mm1(t-1) | relu(t-1, scalar) |
        # mm2(t-3) | per-4-tile-quad bias+sigmoid | per-16-tile epilogue.
        tiles = [None] * N_TILES

        def st_dma(t):
            bt = t // TILES_PER_BATCH
            tt = t % TILES_PER_BATCH
            if t % 4 == 0:
                xt = xtp.tile([128, 4, 4, TILE_ROWS], fp8, name="xt")
                src = xT_d[t // 4].rearrange("p u (c r) -> p u c r", c=4)
                if t == 0:
                    # x bytes first; weights issue after the first 512 KB
                    nc.sync.dma_start(xt[:, 0:2], src[:, 0:2])
                    load_weights()
                    nc.sync.dma_start(xt[:, 2:4], src[:, 2:4])
                elif t in (4, 8):
                    # warmup: finer halves so early tiles unblock sooner
                    nc.sync.dma_start(xt[:, 0:2], src[:, 0:2])
                    nc.sync.dma_start(xt[:, 2:4], src[:, 2:4])
                else:
                    nc.sync.dma_start(xt[:], src)
            if tt == 0:
                # one merged per-batch tile: E (512) | pr (256) | ob (128)
                Bt = epool.tile([128, 896], f32, name="Bt")
                E = Bt[:, 0:512]
                pr = Bt[:, 512:768]
                ob = Bt[:, 768:896]
                nc.sync.dma_start(pr, probs_d[bt])
                for u in range(TILES_PER_BATCH):
                    tiles[bt * TILES_PER_BATCH + u] = {"E": E, "pr": pr, "ob": ob}
            if t % 4 == 0:
                for u in range(4):
                    tiles[t + u]["xt"] = xt
                    tiles[t + u]["xu"] = u

        def st_mm1(t):
            ti = tiles[t]
            u = ti["xu"]
            pm1 = pm1p.tile([128, TILE_ROWS], f32, name="pm1")
            for k in range(4):
                nc.tensor.matmul(pm1[:], W["w13"][:, k, :], ti["xt"][:, u, k, :],
                                 start=(k == 0), stop=(k == 3))
            ti["pm1"] = pm1

        def st_relu(t):
            ti = tiles[t]
            H = hp.tile([128, TILE_ROWS], bf16, name="H")
            nc.scalar.activation(H[:], ti["pm1"][:], AF.Relu, bias=W["b13"][:])
            ti["H"] = H
            ti["pm1"] = None

        def st_mm2(t):
            ti = tiles[t]
            if t % 4 == 0:
                ti["pm2"] = pm2p.tile([128, 128], f32, name="pm2")
                for u in range(1, 4):
                    if t + u < N_TILES:
                        tiles[t + u]["pm2"] = ti["pm2"]
            q = (t % 4) * 32
            for g in range(4):
                nc.tensor.matmul(ti["pm2"][:, q + g * 8:q + (g + 1) * 8],
                                 ti["H"][:, g * 128:(g + 1) * 128], W["wcat"][:],
                                 start=True, stop=True)

        def st_quad(t):
            # after mm2 of quad ending at tile t=4q+3: bias+sigmoid the quad
            ti = tiles[t]
            qq = (t % TILES_PER_BATCH) // 4
            esl = ti["E"][:, qq * 128:(qq + 1) * 128]
            nc.vector.tensor_tensor(esl, ti["pm2"][:], W["b24"][:], OP.add)
            nc.scalar.activation(esl, esl, AF.Sigmoid)
            ti["pm2"] = None

        def st_epiA(t):
            ti = tiles[t]
            Eb = ti["E"].rearrange("q (t e) -> q t e", e=32)
            sc = tpool.tile([128, 2 * (5 * TG + 16) + TG], f32, name="sc")
            rs0 = epi_chain(nc.vector, Eb, ti["pr"], sc, 0, True)
            rs1 = epi_chain(nc.vector, Eb, ti["pr"], sc, 1, True)
            dd = sc[:, 2 * (5 * TG + 16):]
            nc.vector.tensor_tensor(dd, rs0, rs1, OP.subtract)
            ti["dd"] = dd

        def st_epiB(t):
            ti = tiles[t]
            bt = t // TILES_PER_BATCH
            ob = ti["ob"]
            O4 = ob.rearrange("q (tg k) -> q tg k", k=2)
            nc.scalar.activation(O4[:, :, 0], ti["dd"], AF.Sigmoid)
            nc.scalar.activation(O4[:, :, 1], ti["dd"], AF.Sigmoid, scale=-1.0)
            nc.sync.dma_start(out_d[bt], ob)
            for u in range(TILES_PER_BATCH):
                tiles[bt * TILES_PER_BATCH + u] = None

        LAG2 = 3   # mm2 lag
        LAGQ = 8   # quad bias+sigmoid lag
        LAGA = 10  # epilogue vector-chain lag (from batch-end tile)
        LAGB = 12  # epilogue sigmoid+out lag
        for t in range(N_TILES + LAGB):
            if t < N_TILES:
                st_dma(t)
            if 0 <= t - 1 < N_TILES:
                st_mm1(t - 1)
                st_relu(t - 1)
            if 0 <= t - LAG2 < N_TILES:
                st_mm2(t - LAG2)
            if 0 <= t - LAGQ < N_TILES and (t - LAGQ) % 4 == 3:
                st_quad(t - LAGQ)
            if 0 <= t - LAGA < N_TILES and (t - LAGA) % TILES_PER_BATCH == 15:
                st_epiA(t - LAGA)
            if 0 <= t - LAGB < N_TILES and (t - LAGB) % TILES_PER_BATCH == 15:
                st_epiB(t - LAGB)

    nc.compile()
    return nc


def _get_nc():
    if "nc" not in _CACHE:
        _CACHE["nc"] = _build()
    return _CACHE["nc"]


def make_in_maps(probs, fuzzy_features, W1, b1, W2, b2, W3, b3, W4, b4):
    x = np.asarray(fuzzy_features, dtype=np.float32)
    pr = np.asarray(probs, dtype=np.float32).reshape(B, 4)
    W1 = np.asarray(W1, np.float32); b1 = np.asarray(b1, np.float32)
    W2 = np.asarray(W2, np.float32); b2 = np.asarray(b2, np.float32)
    W3 = np.asarray(W3, np.float32); b3 = np.asarray(b3, np.float32)
    W4 = np.asarray(W4, np.float32); b4 = np.asarray(b4, np.float32)

    w13 = np.ascontiguousarray(np.concatenate([W1, W3], axis=1)).astype(ml_dtypes.bfloat16)
    wcat = np.zeros((128, 8), np.float32)
    wcat[0:64, 0:4] = W2
    wcat[64:128, 4:6] = W4
    wcat = wcat.astype(ml_dtypes.bfloat16)
    b13 = np.concatenate([b1, b3]).reshape(128, 1).astype(np.float32)
    pat = np.concatenate([b2, b4, np.zeros(2, np.float32)])             # [8]
    b24 = np.ascontiguousarray(np.tile(pat, (128, 16))).astype(np.float32)

    xbf = x.astype(ml_dtypes.float8_e4m3fn)
    in_maps = []
    for c in range(N_CORES):
        xc = xbf[c * R:(c + 1) * R]
        # xT[g, p, u, kc*512 + j] = x[(2g+u)*512 + j, kc*128 + p]
        xT = np.ascontiguousarray(
            xc.reshape(N_TILES // 4, 4, TILE_ROWS, 4, 128).transpose(0, 4, 1, 3, 2)
        ).reshape(N_TILES // 4, 128, 4, 4 * TILE_ROWS)
        prc = pr[c * R:(c + 1) * R]
        # pr5[bt, p, t, g, i] = probs[bt*8192 + t*512 + g*128 + p, i]
        pr5 = np.ascontiguousarray(
            prc.reshape(N_BATCH, TILES_PER_BATCH, 4, 128, 4).transpose(0, 3, 1, 2, 4)
        ).reshape(N_BATCH, 128, TG * 4)
        in_maps.append({
            "xT": xT, "probs": pr5,
            "w13": w13, "wcat": wcat, "b13": b13, "b24": b24,
        })
    return in_maps


def unpack_out(results):
    outs = []
    for c in range(N_CORES):
        o = results[c]["out"].reshape(N_BATCH, 128, TILES_PER_BATCH, 4, 2)
        # out row bt*8192 + t*512 + g*128 + p
        outs.append(o.transpose(0, 2, 3, 1, 4).reshape(R, 2))
    return np.concatenate(outs, axis=0)


def kernel(probs, fuzzy_features, W1, b1, W2, b2, W3, b3, W4, b4, **kwargs):
    nc = _get_nc()
    in_maps = make_in_maps(probs, fuzzy_features, W1, b1, W2, b2, W3, b3, W4, b4)
    res = bass_utils.run_bass_kernel_spmd(nc, in_maps, core_ids=list(range(N_CORES)))
    return unpack_out(res.results)


# revision 48
# speedup vs baseline: 1.0038x; 1.0038x over previous
"""Trainium2 Bass kernel for nn_MF2Net (two tiny MLPs + Choquet integral + softmax).

Strategy: pure data parallel over the batch dim (8 NeuronCores x 32768 rows).
Host pre-packs x as fp8-e4m3 in feature-major (transposed) 4-tile-group
layout (the two MLP input matmuls dominate and tolerate fp8: measured
3.0e-3 output rel err vs the 2e-2 gate), so the device streams contiguous
1 MB groups on a single HWDGE queue and feeds the PE with zero transposes
and zero PSUM->SBUF copies:
  - DMA xt [128 feat_p, 4 tiles, 4 kc, 512 rows] fp8 per group
  - mm1: pm1[128 hid, 512 r] = sum_kc w13(bf16)[:,kc,:]^T @ xt[:,u,kc,:]
  - ACT: H = relu(pm1 + b13) -> bf16
  - mm2: per 128-row group g: pm2q[:, (u%4)*32+g*8 ...] = H-chunk^T @ wcat
  - per 4-tile quad: DVE +b24, ACT sigmoid into E
  - per 16-tile batch: Choquet combine + softmax-as-sigmoid (DVE chain),
    output written as [128, 128] f32, host un-permutes.
Software-pipelined with stage lags (mm1/relu lag 1, mm2 lag 3, quad lag 8,
epilogue lags 10/12) so the PE stays busy and the sync queue (DMA issues +
semaphore broadcasts) never head-of-line blocks; E/pr/ob live in one merged
per-batch tile to cut semaphore count. Steady state ~0.96 us per 512-row
tile, bound by the tensor+scalar chain, DMA well below its ~400 GB/s/core
ceiling.
"""
import numpy as np
import ml_dtypes
from contextlib import ExitStack

import concourse.bass as bass
import concourse.bacc as bacc
import concourse.tile as tile
import concourse.mybir as mybir
from concourse import bass_utils

N_CORES = 8
B = 262144
D = 512
R = B // N_CORES            # rows per core = 32768
TILE_ROWS = 512
N_TILES = R // TILE_ROWS    # 64
TILES_PER_BATCH = 16
BATCH_ROWS = TILE_ROWS * TILES_PER_BATCH   # 8192
N_BATCH = N_TILES // TILES_PER_BATCH       # 4
TG = TILES_PER_BATCH * 4                   # 64 (tile16, group) pairs per batch

_CACHE = {}


def _build():
    f32 = mybir.dt.float32
    bf16 = mybir.dt.bfloat16
    AF = mybir.ActivationFunctionType
    OP = mybir.AluOpType

    nc = bacc.Bacc("TRN2", target_bir_lowering=False, debug=False,
                   enable_asserts=False, num_devices=N_CORES)
    fp8 = mybir.dt.float8e4
    # x transposed+tiled on host, 4-tile chunks: [chunk, feat_part, tile, kc*rows]
    xT_d = nc.dram_tensor("xT", [N_TILES // 4, 128, 4, 4 * TILE_ROWS], fp8,
                          kind="ExternalInput").ap()
    # probs packed on host: [batch, row_part, tile16*group*4] f32
    probs_d = nc.dram_tensor("probs", [N_BATCH, 128, TG * 4], f32,
                             kind="ExternalInput").ap()
    w13_d = nc.dram_tensor("w13", [D, 128], bf16, kind="ExternalInput").ap()
    wcat_d = nc.dram_tensor("wcat", [128, 8], bf16, kind="ExternalInput").ap()
    b13_d = nc.dram_tensor("b13", [128, 1], f32, kind="ExternalInput").ap()
    b24_d = nc.dram_tensor("b24", [128, 128], f32, kind="ExternalInput").ap()
    # out packed: [batch, row_part, tile16*group*2] f32 (host un-permutes)
    out_d = nc.dram_tensor("out", [N_BATCH, 128, TG * 2], f32,
                           kind="ExternalOutput").ap()

    with tile.TileContext(nc) as tc, ExitStack() as ctx:
        wpool = ctx.enter_context(tc.tile_pool(name="w", bufs=1))
        xtp = ctx.enter_context(tc.tile_pool(name="xt", bufs=6))
        hp = ctx.enter_context(tc.tile_pool(name="h", bufs=4))
        epool = ctx.enter_context(tc.tile_pool(name="e", bufs=2))
        tpool = ctx.enter_context(tc.tile_pool(name="t", bufs=2))
        pm1p = ctx.enter_context(tc.tile_pool(name="pm1", bufs=4, space="PSUM"))
        pm2p = ctx.enter_context(tc.tile_pool(name="pm2", bufs=3, space="PSUM"))
        pmdp = ctx.enter_context(tc.tile_pool(name="pmd", bufs=1, space="PSUM"))

        W = {}

        def load_weights():
            W["w13"] = wpool.tile([128, 4, 128], bf16, name="w13sb")
            nc.sync.dma_start(W["w13"][:], w13_d.rearrange("(c p) h -> p c h", p=128))
            W["wcat"] = wpool.tile([128, 8], bf16, name="wcatsb")
            nc.sync.dma_start(W["wcat"][:], wcat_d)
            W["b13"] = wpool.tile([128, 1], f32, name="b13sb")
            nc.sync.dma_start(W["b13"][:], b13_d)
            W["b24"] = wpool.tile([128, 128], f32, name="b24sb")
            nc.sync.dma_start(W["b24"][:], b24_d)

        def epi_chain(eng, Eb, prb, sc, kc, use_pred):
            E4 = Eb.rearrange("q t (g c) -> q (t g) c", c=8)
            P4 = prb.rearrange("q (tg i) -> q tg i", i=4)
            mu1 = E4[:, :, 0 + kc]
            mu2 = E4[:, :, 2 + kc]
            inc = E4[:, :, 4 + kc]
            p0 = P4[:, :, 0 + kc]
            p1 = P4[:, :, 2 + kc]
            o = kc * (5 * TG + 16)
            mx = sc[:, o:o + TG]
            dm = sc[:, o + TG:o + 2 * TG]
            r1 = sc[:, o + 2 * TG:o + 3 * TG]
            r2 = sc[:, o + 3 * TG:o + 4 * TG]
            rs = sc[:, o + 4 * TG:o + 5 * TG]
            msk = sc[:, o + 5 * TG:o + 5 * TG + 16].bitcast(mybir.dt.uint8)
            eng.tensor_tensor(mx, mu1, mu2, OP.max)
            eng.tensor_tensor(mx, mx, inc, OP.add)
            eng.tensor_scalar_min(mx, mx, 1.0)
            eng.tensor_tensor(dm, p1, p0, OP.subtract)
            eng.tensor_tensor(dm, dm, mx, OP.mult)
            eng.tensor_tensor(r1, p0, mu1, OP.mult)
            eng.tensor_tensor(r1, r1, dm, OP.add)
            eng.tensor_tensor(r2, p1, mu2, OP.mult)
            eng.tensor_tensor(r2, r2, dm, OP.subtract)
            eng.tensor_tensor(msk, p0, p1, OP.is_le)
            eng.tensor_copy(rs, r2)
            eng.copy_predicated(rs, msk, r1)
            return rs

        # Software-pipelined stages (lags keep the PE continuously busy so it
        # ramps to its max p-state): dma(t) | mm1(t-1) | relu(t-1, scalar) |
        # mm2(t-3) | per-4-tile-quad bias+sigmoid | per-16-tile epilogue.
        tiles = [None] * N_TILES

        def st_dma(t):
            bt = t // TILES_PER_BATCH
            tt = t % TILES_PER_BATCH
            if t % 4 == 0:
                xt = xtp.tile([128, 4, 4, TILE_ROWS], fp8, name="xt")
                src = xT_d[t // 4].rearrange("p u (c r) -> p u c r", c=4)
                if t == 0:
                    # x bytes first; weights issue after the first 512 KB
                    nc.sync.dma_start(xt[:, 0:2], src[:, 0:2])
                    load_weights()
                    nc.sync.dma_start(xt[:, 2:4], src[:, 2:4])
                    # pre-warm the PE clock (DVFS ramps only while busy):
                    # dummy matmuls on memset tiles while x streams in
                    dw = wpool.tile([128, 128], bf16, name="dw")
                    dx = wpool.tile([128, 512], fp8, name="dx")
                    nc.vector.memset(dw[:], 1.0)
                    nc.vector.memset(dx[:], 1.0)
                    pmd = pmdp.tile([128, 512], f32, name="pmd")
                    for _ in range(10):
                        nc.tensor.matmul(pmd[:], dw[:], dx[:],
                                         start=True, stop=True)
                else:
                    nc.sync.dma_start(xt[:], src)
            if tt == 0:
                # one merged per-batch tile: E (512) | pr (256) | ob (128)
                Bt = epool.tile([128, 896], f32, name="Bt")
                E = Bt[:, 0:512]
                pr = Bt[:, 512:768]
                ob = Bt[:, 768:896]
                nc.sync.dma_start(pr, probs_d[bt])
                for u in range(TILES_PER_BATCH):
                    tiles[bt * TILES_PER_BATCH + u] = {"E": E, "pr": pr, "ob": ob}
            if t % 4 == 0:
                for u in range(4):
                    tiles[t + u]["xt"] = xt
                    tiles[t + u]["xu"] = u

        def st_mm1(t):
            ti = tiles[t]
            u = ti["xu"]
            pm1 = pm1p.tile([128, TILE_ROWS], f32, name="pm1")
            for k in range(4):
                nc.tensor.matmul(pm1[:], W["w13"][:, k, :], ti["xt"][:, u, k, :],
                                 start=(k == 0), stop=(k == 3))
            ti["pm1"] = pm1

        def st_relu(t):
            ti = tiles[t]
            H = hp.tile([128, TILE_ROWS], bf16, name="H")
            nc.scalar.activation(H[:], ti["pm1"][:], AF.Relu, bias=W["b13"][:])
            ti["H"] = H
            ti["pm1"] = None

        def st_mm2(t):
            ti = tiles[t]
            if t % 4 == 0:
                ti["pm2"] = pm2p.tile([128, 128], f32, name="pm2")
                for u in range(1, 4):
                    if t + u < N_TILES:
                        tiles[t + u]["pm2"] = ti["pm2"]
            q = (t % 4) * 32
            for g in range(4):
                nc.tensor.matmul(ti["pm2"][:, q + g * 8:q + (g + 1) * 8],
                                 ti["H"][:, g * 128:(g + 1) * 128], W["wcat"][:],
                                 start=True, stop=True)

        def st_quad(t):
            # after mm2 of quad ending at tile t=4q+3: bias+sigmoid the quad
            ti = tiles[t]
            qq = (t % TILES_PER_BATCH) // 4
            esl = ti["E"][:, qq * 128:(qq + 1) * 128]
            nc.vector.tensor_tensor(esl, ti["pm2"][:], W["b24"][:], OP.add)
            nc.scalar.activation(esl, esl, AF.Sigmoid)
            ti["pm2"] = None

        def st_epiA(t):
            ti = tiles[t]
            Eb = ti["E"].rearrange("q (t e) -> q t e", e=32)
            sc = tpool.tile([128, 2 * (5 * TG + 16) + TG], f32, name="sc")
            rs0 = epi_chain(nc.vector, Eb, ti["pr"], sc, 0, True)
            rs1 = epi_chain(nc.vector, Eb, ti["pr"], sc, 1, True)
            dd = sc[:, 2 * (5 * TG + 16):]
            nc.vector.tensor_tensor(dd, rs0, rs1, OP.subtract)
            ti["dd"] = dd

        def st_epiB(t):
            ti = tiles[t]
            bt = t // TILES_PER_BATCH
            ob = ti["ob"]
            O4 = ob.rearrange("q (tg k) -> q tg k", k=2)
            nc.scalar.activation(O4[:, :, 0], ti["dd"], AF.Sigmoid)
            nc.scalar.activation(O4[:, :, 1], ti["dd"], AF.Sigmoid, scale=-1.0)
            nc.sync.dma_start(out_d[bt], ob)
            for u in range(TILES_PER_BATCH):
                tiles[bt * TILES_PER_BATCH + u] = None

        LAG2 = 3   # mm2 lag
        LAGQ = 8   # quad bias+sigmoid lag
        LAGA = 10  # epilogue vector-chain lag (from batch-end tile)
        LAGB = 12  # epilogue sigmoid+out lag
        for t in range(N_TILES + LAGB):
            if t < N_TILES:
                st_dma(t)
            if 0 <= t - 1 < N_TILES:
                st_mm1(t - 1)
                st_relu(t - 1)
            if 0 <= t - LAG2 < N_TILES:
                st_mm2(t - LAG2)
            if 0 <= t - LAGQ < N_TILES and (t - LAGQ) % 4 == 3:
                st_quad(t - LAGQ)
            if 0 <= t - LAGA < N_TILES and (t - LAGA) % TILES_PER_BATCH == 15:
                st_epiA(t - LAGA)
            if 0 <= t - LAGB < N_TILES and (t - LAGB) % TILES_PER_BATCH == 15:
                st_epiB(t - LAGB)

    nc.compile()
    return nc


def _get_nc():
    if "nc" not in _CACHE:
        _CACHE["nc"] = _build()
    return _CACHE["nc"]


def make_in_maps(probs, fuzzy_features, W1, b1, W2, b2, W3, b3, W4, b4):
    x = np.asarray(fuzzy_features, dtype=np.float32)
    pr = np.asarray(probs, dtype=np.float32).reshape(B, 4)
    W1 = np.asarray(W1, np.float32); b1 = np.asarray(b1, np.float32)
    W2 = np.asarray(W2, np.float32); b2 = np.asarray(b2, np.float32)
    W3 = np.asarray(W3, np.float32); b3 = np.asarray(b3, np.float32)
    W4 = np.asarray(W4, np.float32); b4 = np.asarray(b4, np.float32)

    w13 = np.ascontiguousarray(np.concatenate([W1, W3], axis=1)).astype(ml_dtypes.bfloat16)
    wcat = np.zeros((128, 8), np.float32)
    wcat[0:64, 0:4] = W2
    wcat[64:128, 4:6] = W4
    wcat = wcat.astype(ml_dtypes.bfloat16)
    b13 = np.concatenate([b1, b3]).reshape(128, 1).astype(np.float32)
    pat = np.concatenate([b2, b4, np.zeros(2, np.float32)])             # [8]
    b24 = np.ascontiguousarray(np.tile(pat, (128, 16))).astype(np.float32)

    xbf = x.astype(ml_dtypes.float8_e4m3fn)
    in_maps = []
    for c in range(N_CORES):
        xc = xbf[c * R:(c + 1) * R]
        # xT[g, p, u, kc*512 + j] = x[(2g+u)*512 + j, kc*128 + p]
        xT = np.ascontiguousarray(
            xc.reshape(N_TILES // 4, 4, TILE_ROWS, 4, 128).transpose(0, 4, 1, 3, 2)
        ).reshape(N_TILES // 4, 128, 4, 4 * TILE_ROWS)
        prc = pr[c * R:(c + 1) * R]
        # pr5[bt, p, t, g, i] = probs[bt*8192 + t*512 + g*128 + p, i]
        pr5 = np.ascontiguousarray(
            prc.reshape(N_BATCH, TILES_PER_BATCH, 4, 128, 4).transpose(0, 3, 1, 2, 4)
        ).reshape(N_BATCH, 128, TG * 4)
        in_maps.append({
            "xT": xT, "probs": pr5,
            "w13": w13, "wcat": wcat, "b13": b13, "b24": b24,
        })
    return in_maps


def unpack_out(results):
    outs = []
    for c in range(N_CORES):
        o = results[c]["out"].reshape(N_BATCH, 128, TILES_PER_BATCH, 4, 2)
        # out row bt*8192 + t*512 + g*128 + p
        outs.append(o.transpose(0, 2, 3, 1, 4).reshape(R, 2))
    return np.concatenate(outs, axis=0)


def kernel(probs, fuzzy_features, W1, b1, W2, b2, W3, b3, W4, b4, **kwargs):
    nc = _get_nc()
    in_maps = make_in_maps(probs, fuzzy_features, W1, b1, W2, b2, W3, b3, W4, b4)
    res = bass_utils.run_bass_kernel_spmd(nc, in_maps, core_ids=list(range(N_CORES)))
    return unpack_out(res.results)
